# revision 92
# baseline (speedup 1.0000x reference)
"""Deformable head attention kernel for 8 Trainium2 NeuronCores.

Sharding: core i handles batch b = i//2 and head-group hg = i%2 (heads
4*hg..4*hg+3, all 4096 queries). The reference's final reshape maps output
pixel p' to head p'//512's features of queries 8t..8t+7 (t = p' % 512), so a
head-group owns output rows [hg*32, hg*32+32) exactly -- fully local per core.

Per core (v3, overlap-oriented; ~650us vs the 1107us v1 baseline):
  - bf16 host-prepped inputs/weights (halves the key loads, 1-cycle matmuls)
  - per-(head,scale) quad maps (2x2x32ch tokens, 256B bf16) in DRAM; key
    slabs prefetched one ahead so the write stream never stalls; pad regions
    zero-filled byte-disjointly (no WAW ordering needed) and deferred off the
    early HWDGE window
  - gather indices built with PE "selection matmuls" (ident column slices
    fold partition ph*16+r to idx row r with ph in the free dim — the SWDGE
    16-wrap layout) instead of 256 small DMAs; idx rows replicated to all
    eight 16-partition groups per q-half (the gather engines read each group)
  - coordinate/weight math split x-on-DVE / y-on-Pool, quarter-tiled for SBUF
  - SWDGE dma_gather fetches one quad per sample (the 22.76ns/descriptor
    cost-model floor); the weighted reduce runs on DVE so the 32-gather train
    is purely DMA-bound at ~99% occupancy
  - output projection straight from SBUF: per (head, q-half), feat slices are
    PE-transposed to TS[d, c, p] as their last scale lands, then eight
    32-partition Wm row-block matmuls (column slice p=e::8) accumulate the
    e-partials in PSUM — no DRAM scramble round-trip and no tail barrier
"""
import os
import numpy as np
import ml_dtypes
from contextlib import ExitStack

BFNP = ml_dtypes.bfloat16

import concourse.bass as bass
import concourse.tile as tile
from concourse import bacc, mybir
from concourse.bass_utils import run_bass_kernel_spmd
from concourse.masks import make_identity

F32 = mybir.dt.float32
F32R = mybir.dt.float32r
I32 = mybir.dt.int32
I16 = mybir.dt.int16
BF16 = mybir.dt.bfloat16
OP = mybir.AluOpType
AF = mybir.ActivationFunctionType

HEADS, KPTS, SCALES, D = 8, 4, 4, 256
DK = D // HEADS              # 32
HL = 4                       # heads per core
B, H, W = 4, 64, 64
Q = 4096                     # queries per core (full image)
QC = Q // 128                # 32 q-chunks
HW_SIZES = [(16, 16), (32, 32), (64, 64), (128, 128)]
POS = [h * w for h, w in HW_SIZES]
TCAP = [p + w + 4 for p, (h, w) in zip(POS, HW_SIZES)]
NCORES = 8
NIDX = KPTS * 2048           # 8192 gather indices per (head, scale, q-half)
LS = 2048                    # key-projection slab (tokens)

_cache = {}


def _build():
    nc = bacc.Bacc("TRN2", target_bir_lowering=False, debug=False)

    d_queryT = nc.dram_tensor("queryT", [2, 128, Q], BF16, kind="ExternalInput")
    d_keysT = [nc.dram_tensor(f"keysT{l}", [2, 128, POS[l]], BF16,
                              kind="ExternalInput")
               for l in range(SCALES)]
    d_cblk = nc.dram_tensor("cblk", [128, 837], F32, kind="ExternalInput")
    d_Wq = nc.dram_tensor("Wq", [2, 128, D], BF16, kind="ExternalInput")
    d_Wk = nc.dram_tensor("Wk", [2, 128, 128], BF16, kind="ExternalInput")
    d_Woff = nc.dram_tensor("WoffP", [2, 128, 128], BF16, kind="ExternalInput")
    d_WA = nc.dram_tensor("WA", [2, 128, 64], BF16, kind="ExternalInput")
    d_Wm = nc.dram_tensor("WmE", [32, 8, D], BF16, kind="ExternalInput")
    cnames = ["wl_t", "wlm1_t", "wlm2_t", "wlp1_t", "hlm1_t", "hlm2_t",
              "hl_t", "cofx_t", "cofy_t"]

    d_out = nc.dram_tensor("outT", [2, 128, 2048], F32, kind="ExternalOutput")
    DBG = bool(int(os.environ.get("KDBG", "0")))
    if DBG:
        d_dbg_G = nc.dram_tensor("dbg_G", [128, 8192], BF16,
                                 kind="ExternalOutput")
        d_dbg_M = nc.dram_tensor("dbg_M", [128, 8192], BF16,
                                 kind="ExternalOutput")
        d_dbg_idx = nc.dram_tensor("dbg_idx", [128, 16384], I16,
                                   kind="ExternalOutput")
        d_dbg_feat = nc.dram_tensor("dbg_feat", [128, 4096], F32,
                                    kind="ExternalOutput")
        d_dbg_w4 = nc.dram_tensor("dbg_w4", [128, 16384], BF16,
                                  kind="ExternalOutput")
        d_dbg_map1 = nc.dram_tensor("dbg_map1", [HL, TCAP[1], 128], BF16,
                                    kind="ExternalOutput")
        d_dbg_map0 = nc.dram_tensor("dbg_map0", [HL, TCAP[0], 128], BF16,
                                    kind="ExternalOutput")

    with tile.TileContext(nc) as tc, ExitStack() as ctx:
        wpool = ctx.enter_context(tc.tile_pool(name="weights", bufs=1))
        ppool = ctx.enter_context(tc.tile_pool(name="persist", bufs=1))
        psum = ctx.enter_context(tc.tile_pool(name="psum", bufs=4, space="PSUM"))
        psumb = ctx.enter_context(tc.tile_pool(name="psumb", bufs=2, space="PSUM"))
        dram = ctx.enter_context(tc.tile_pool(name="dramp", bufs=1, space="DRAM"))

        d_map = [dram.tile([HL, TCAP[l], 128], BF16, tag=f"map{l}", name=f"map{l}")
                 for l in range(SCALES)]
        d_featD = dram.tile([2048, 256], F32, tag="featD", name="featD")

        def load2(d, n, nm, dt=F32):
            t2 = wpool.tile([128, 2, n], dt, tag=nm, name=nm)
            nc.sync.dma_start(t2[:], d[:].rearrange("k p n -> p k n"))
            return [t2[:, 0], t2[:, 1]]

        def load1(d, shape, nm):
            t = wpool.tile(shape, F32, tag=nm, name=nm)
            nc.sync.dma_start(t[:], d[:])
            return t

        Wq = load2(d_Wq, D, "Wq", BF16); Wk = load2(d_Wk, 128, "Wk", BF16)
        Woff = load2(d_Woff, 128, "Woff", BF16)
        WA = load2(d_WA, 64, "WA", BF16)
        WmE = wpool.tile([128, 8, D], BF16, tag="WmE", name="WmE")
        nc.sync.dma_start(WmE[0:32], d_Wm[:])
        cblk = load1(d_cblk, [128, 837], "cblk")
        C = {n: cblk[:, i * 64:(i + 1) * 64] for i, n in enumerate(cnames)}
        boffE = cblk[:, 576:704]
        bAE = cblk[:, 704:768]
        refx = cblk[:, 768:800]
        refy = cblk[:, 800:832]
        bq = [cblk[:, 832 + i:833 + i] for i in range(2)]
        bm = [cblk[:, 834 + i:835 + i] for i in range(2)]
        bk = cblk[:, 836:837]
        identF = wpool.tile([128, 128], F32, tag="identF", name="identF")
        make_identity(nc, identF[:])
        identB = wpool.tile([128, 128], BF16, tag="identB", name="identB")
        make_identity(nc, identB[:])
        zt = wpool.tile([128, HL, 32], BF16, tag="zt", name="zt")
        nc.vector.memset(zt[:], 0)

        # persistent state
        W4 = ppool.tile([128, HL, SCALES, KPTS, QC, 4, 2], BF16, tag="W4",
                        name="W4")
        feat = ppool.tile([128, QC, HL, DK], F32, tag="feat", name="feat")
        IDX16 = ppool.tile([128, 2, HL, SCALES, KPTS, 16, 8], I16,
                           tag="IDX16", name="IDX16")
        nc.vector.memset(IDX16[:], 0)
        nc.vector.memset(feat[:], 0)

        # =========== quad map construction (phase D) ===========
        mp = tc.alloc_tile_pool(name="mapp", bufs=1)
        CORNERS = [(0, 0), (0, 1), (1, 0), (1, 1)]

        def zero_edges(l):
            """Fill every map byte not covered by a corner write with zeros.
            Corner (cy,cx) covers entries [base, base+POS) at entry-offset
            (cy*2+cx)*32; zero the complement so reads never see garbage."""
            mapl = d_map[l]
            hl_, wl = HW_SIZES[l]
            for cy, cx in CORNERS:
                base = wl + 1 - cy * wl - cx
                off = (cy * 2 + cx) * 32
                for t0, t1 in ((0, base), (base + POS[l], TCAP[l])):
                    t = t0
                    while t < t1:
                        n = min(128, t1 - t)
                        dst = bass.AP(
                            mapl.tensor,
                            mapl.offset + t * 128 + off,
                            ((128, n), (TCAP[l] * 128, HL), (1, 32)))
                        nc.scalar.dma_start(dst, zt[0:n, :, :])
                        t += n

        def load_slab(l, sl, slab):
            kin = [mp.tile([128, slab], BF16, tag=f"kin{i}", name=f"kin{i}",
                           bufs=2) for i in range(2)]
            for i in range(2):
                nc.sync.dma_start(kin[i][:],
                                  d_keysT[l][i, :, sl * slab:(sl + 1) * slab])
            return kin

        def build_map(l):
            hl_, wl = HW_SIZES[l]
            mapl = d_map[l]
            slab = min(LS, POS[l])
            nslab = POS[l] // slab
            kin = load_slab(l, 0, slab)
            for sl in range(nslab):
                kin_next = load_slab(l, sl + 1, slab) if sl + 1 < nslab else None
                kfs = mp.tile([128, slab], BF16, tag="kfs", name="kfs", bufs=2)
                for c0 in range(0, slab, 512):
                    cw = min(512, slab - c0)
                    ps = psum.tile([128, 512], F32, tag="mm", name="mm")
                    for k in range(2):
                        nc.tensor.matmul(ps[:, 0:cw], Wk[k][:],
                                         kin[k][:, c0:c0 + cw],
                                         start=(k == 0), stop=(k == 1))
                    nc.scalar.activation(kfs[:, c0:c0 + cw], ps[:, 0:cw],
                                         AF.Identity, bias=bk[:], scale=1.0)
                nsub = slab // 128
                stg = mp.tile([128, nsub, 128], BF16, tag="stg", name="stg",
                              bufs=3)
                for g0 in range(0, nsub, 4):
                    gw = min(4, nsub - g0)
                    pt = psumb.tile([128, 512], BF16, tag="tp", name="tp")
                    for j in range(gw):
                        nc.tensor.transpose(
                            pt[:, j * 128:(j + 1) * 128],
                            kfs[:, (g0 + j) * 128:(g0 + j + 1) * 128],
                            identB[:])
                    nc.scalar.activation(
                        stg[:, g0:g0 + gw].rearrange("p a e -> p (a e)"),
                        pt[:, 0:gw * 128], AF.Copy)
                src = stg[:].rearrange("p a (h e) -> p h a e", e=DK)
                for cy, cx in CORNERS:
                    base = wl + 1 - cy * wl - cx
                    off = (base + sl * slab) * 128 + (cy * 2 + cx) * DK
                    for hh in range(HL):
                        dst = bass.AP(
                            mapl.tensor,
                            mapl.offset + off + hh * TCAP[l] * 128,
                            ((128, 128), (128 * 128, nsub), (1, DK)))
                        nc.sync.dma_start(dst, src[:, hh])
                kin = kin_next

        # queryT loads ride the Act queue so the SP/map DMA stream is unbroken
        pbc = tc.alloc_tile_pool(name="pbc", bufs=1)
        Aw = pbc.tile([128, QC, 64], F32, tag="Aw", name="Aw")
        offxy = pbc.tile([128, QC, 128], F32, tag="offxy", name="offxy")
        proj = tc.alloc_tile_pool(name="proj", bufs=1)
        queryT = [proj.tile([128, Q], BF16, tag=f"qin{i}", name=f"qin{i}")
                  for i in range(2)]
        for i in range(2):
            nc.scalar.dma_start(queryT[i][:], d_queryT[i])

        for l in range(3):
            build_map(l)

        # =========== phase B: q / offset / attention projections ===========
        if True:
            qT = [proj.tile([128, Q], BF16, tag=f"qT{i}", name=f"qT{i}")
                  for i in range(2)]
            for m in range(2):
                for n in range(Q // 512):
                    ps = psum.tile([128, 512], F32, tag="mm", name="mm")
                    for k in range(2):
                        nc.tensor.matmul(ps[:], Wq[k][:, m * 128:(m + 1) * 128],
                                         queryT[k][:, n * 512:(n + 1) * 512],
                                         start=(k == 0), stop=(k == 1))
                    nc.scalar.activation(qT[m][:, n * 512:(n + 1) * 512], ps[:],
                                         AF.Identity, bias=bq[m][:], scale=1.0)
            for c in range(QC):
                ps = psum.tile([128, 512], F32, tag="mm", name="mm")
                for k in range(2):
                    nc.tensor.matmul(ps[:, 0:128], qT[k][:, c * 128:(c + 1) * 128],
                                     Woff[k][:], start=(k == 0), stop=(k == 1))
                nc.scalar.activation(offxy[:, c], ps[:, 0:128], AF.Copy)
                ps2 = psum.tile([128, 512], F32, tag="mm", name="mm")
                for k in range(2):
                    nc.tensor.matmul(ps2[:, 0:64], qT[k][:, c * 128:(c + 1) * 128],
                                     WA[k][:], start=(k == 0), stop=(k == 1))
                nc.scalar.activation(Aw[:, c], ps2[:, 0:64], AF.Copy)
        proj.release()

        # =========== phase C: coords, weights, tokens (x: DVE, y: Pool) ====
        tokp = tc.alloc_tile_pool(name="tokp", bufs=1)
        tokf = [tokp.tile([128, 16, 64], F32, tag=f"tokf{qh}", name=f"tokf{qh}")
                for qh in range(2)]
        NQ = 8                   # q-chunks per coord quarter

        def bhq(t, n=64):
            return t[:].rearrange("p (o f) -> p o f", o=1).broadcast_to([128, NQ, n])

        for qq in range(QC // NQ):
            sl = slice(qq * NQ, (qq + 1) * NQ)
            with tc.tile_pool(name=f"coord{qq}", bufs=1) as cp:
                def ct(tag, dt=F32):
                    return cp.tile([128, NQ, 64], dt, tag=tag, name=tag)

                ix = ct("ix"); iy = ct("iy")
                for c in range(NQ):
                    cc = qq * NQ + c
                    nc.vector.tensor_scalar(ix[:, c], C["wl_t"][:],
                                            refx[:, cc:cc + 1], -0.5,
                                            OP.mult, OP.add)
                    nc.gpsimd.tensor_scalar(iy[:, c], C["hl_t"][:],
                                            refy[:, cc:cc + 1], -0.5,
                                            OP.mult, OP.add)
                oxv = offxy[:, sl, 0:64]
                oyv = offxy[:, sl, 64:128]
                nc.vector.tensor_tensor(oxv, oxv, bhq(boffE[:, 0:64]), OP.add)
                nc.gpsimd.tensor_tensor(oyv, oyv, bhq(boffE[:, 64:128]), OP.add)
                nc.vector.tensor_tensor(oxv, oxv, bhq(C["cofx_t"]), OP.mult)
                nc.gpsimd.tensor_tensor(oyv, oyv, bhq(C["cofy_t"]), OP.mult)
                nc.vector.tensor_tensor(ix[:], ix[:], oxv, OP.add)
                nc.gpsimd.tensor_tensor(iy[:], iy[:], oyv, OP.add)

                def dimchain(eng, ceng, iv, lim1, lim2, pre):
                    # ceng runs the comparison (bitvec) ops: Pool lacks them
                    xm = ct(f"xm{pre}")
                    eng.tensor_scalar(xm[:], iv[:], 0.5, None, OP.subtract)
                    xi = ct(f"xi{pre}", I32)
                    eng.tensor_copy(xi[:], xm[:])      # RNE => floor(iv)
                    x0 = ct(f"x0{pre}")
                    eng.tensor_copy(x0[:], xi[:])
                    w1 = ct(f"w1{pre}")
                    eng.tensor_tensor(w1[:], iv[:], x0[:], OP.subtract)
                    w0 = ct(f"w0{pre}")
                    eng.tensor_scalar(w0[:], w1[:], -1.0, 1.0, OP.mult, OP.add)
                    m = ct(f"m{pre}")
                    mb = ct(f"mb{pre}")
                    ceng.tensor_scalar(m[:], x0[:], 0.0, None, OP.is_ge)
                    ceng.tensor_tensor(mb[:], x0[:], bhq(lim1), OP.is_le)
                    eng.tensor_tensor(m[:], m[:], mb[:], OP.mult)
                    eng.tensor_tensor(w0[:], w0[:], m[:], OP.mult)
                    ceng.tensor_scalar(m[:], x0[:], -1.0, None, OP.is_ge)
                    ceng.tensor_tensor(mb[:], x0[:], bhq(lim2), OP.is_le)
                    eng.tensor_tensor(m[:], m[:], mb[:], OP.mult)
                    eng.tensor_tensor(w1[:], w1[:], m[:], OP.mult)
                    return x0, w0, w1

                x0f, wx0, wx1 = dimchain(nc.vector, nc.vector, ix,
                                         C["wlm1_t"], C["wlm2_t"], "x")
                y0f, wy0, wy1 = dimchain(nc.gpsimd, nc.vector, iy,
                                         C["hlm1_t"], C["hlm2_t"], "y")

                # softmax over (s, k) per head for this q-quarter
                Av = Aw[:, sl]
                nc.vector.tensor_tensor(Av, Av, bhq(bAE), OP.add)
                nc.scalar.activation(Av, Av, AF.Exp)
                Aw4 = Av.rearrange("p c (h s) -> p c h s", s=16)
                ssum = cp.tile([128, NQ, HL], F32, tag="ssum", name="ssum")
                nc.vector.tensor_reduce(ssum[:], Aw4, mybir.AxisListType.X,
                                        OP.add)
                nc.vector.reciprocal(ssum[:], ssum[:])
                rb = ssum[:].rearrange("p c (h o) -> p c h o", o=1) \
                            .broadcast_to([128, NQ, HL, 16])
                nc.vector.tensor_tensor(Aw4, Aw4, rb, OP.mult)

                # combined interp weights
                W4v = W4[:].rearrange("p h s k c f d -> p c (h s k) f d")[:, sl]
                u = ct("mx")      # reuse
                for cy, wyv in ((0, wy0), (1, wy1)):
                    nc.vector.tensor_tensor(u[:], Av, wyv[:], OP.mult)
                    for cx, wxv in ((0, wx0), (1, wx1)):
                        for dup in range(2):
                            eng = nc.vector if (cx + dup) % 2 == 0 else nc.gpsimd
                            eng.tensor_tensor(
                                W4v[:, :, :, cy * 2 + cx, dup],
                                u[:], wxv[:], OP.mult)

                # token index (+wl+1 offset baked in); min/max live on DVE
                nc.vector.tensor_scalar(y0f[:], y0f[:], -1.0, None, OP.max)
                nc.vector.tensor_tensor(y0f[:], y0f[:], bhq(C["hlm1_t"]),
                                        OP.min)
                nc.vector.tensor_scalar(x0f[:], x0f[:], -1.0, None, OP.max)
                nc.vector.tensor_tensor(x0f[:], x0f[:], bhq(C["wl_t"]), OP.min)
                tk = tokf[qq // 2][:, (qq % 2) * NQ:(qq % 2 + 1) * NQ]
                nc.gpsimd.tensor_tensor(tk, y0f[:], bhq(C["wl_t"]), OP.mult)
                nc.gpsimd.tensor_tensor(tk, tk, x0f[:], OP.add)
                nc.gpsimd.tensor_tensor(tk, tk, bhq(C["wlp1_t"]), OP.add)

        build_map(3)
        # pad zero-fill is byte-disjoint from the corner writes, so it can
        # run this late — keeping early HWDGE free for the map pipeline
        for l in range(SCALES):
            zero_edges(l)

        # =========== gather-index build: PE partition fold ===========
        # idx[r, (k, qcl, ph)] = token[p = ph*16 + r]; selection matmul with
        # an identity column slice folds partitions exactly into the SWDGE
        # 16-wrap layout. f32 matmul is exact for these integer magnitudes.
        with tc.tile_pool(name="idxb", bufs=1) as ib:
            for qh in range(2):
                tsrc = tokf[qh][:].rearrange("p c h -> p (c h)")
                for ph in range(8):
                    for ck in range(2):
                        ps = psum.tile([128, 512], F32, tag="mm", name="mm")
                        nc.tensor.matmul(ps[0:16, :],
                                         identF[:, ph * 16:ph * 16 + 16],
                                         tsrc[:, ck * 512:(ck + 1) * 512],
                                         start=True, stop=True)
                        sti = ib.tile([128, 512], I32, tag="sti", name="sti",
                                      bufs=2)
                        nc.vector.tensor_copy(sti[0:16, :], ps[0:16, :])
                        dst = IDX16[0:16, qh, :, :, :, ck * 8:(ck + 1) * 8, ph]
                        src = sti[0:16].bitcast(I16)[:, 0:1024:2] \
                            .rearrange("p (c h s k) -> p h s k c",
                                       c=8, h=HL, s=SCALES)
                        nc.gpsimd.tensor_copy(dst, src)
                # the gather engines read idx from every 16-partition group:
                # replicate rows 0:16 across all 128 as each half finishes
                for d0, n in ((16, 16), (32, 32), (64, 64)):
                    nc.sync.dma_start(IDX16[d0:d0 + n, qh], IDX16[0:n, qh])
        tokp.release()
        pbc.release()
        mp.release()

        # DRAM tiles get no RAW tracking into dma_gather: fence map writes
        # (and IDX16) before the gather train starts.
        tc.strict_bb_all_engine_barrier()

        # =========== phase E: gather + interpolate, per-head output =======
        op = ctx.enter_context(tc.tile_pool(name="outp", bufs=1))
        fD = d_featD[:].rearrange("o c -> (o c)")
        with tc.tile_pool(name="gath", bufs=1) as gp:
            for h in range(HL):
                TS = op.tile([128, QC, 128], BF16, tag="TS", name="TS",
                             bufs=2)
                for l in range(SCALES):
                    for qh in range(2):
                        G = gp.tile([128, KPTS * 16, 128], BF16, tag="G",
                                    name="G", bufs=2)
                        nc.gpsimd.dma_gather(
                            G[:], d_map[l][h],
                            IDX16[:, qh, h, l].rearrange("p k c e -> p (k c e)"),
                            num_idxs=NIDX, num_idxs_reg=NIDX,
                            elem_size=128, elem_step=128, single_packet=False)
                        M = gp.tile([128, KPTS * 16, 128], BF16, tag="M",
                                    name="M", bufs=1)
                        for kk in range(KPTS):
                            wv = W4[:, h, l, kk, qh * 16:(qh + 1) * 16, :, :] \
                                .rearrange("p c f (o d) -> p c f o d", o=1) \
                                .broadcast_to([128, 16, 4, 16, 2])
                            nc.vector.tensor_tensor(
                                M[:, kk * 16:(kk + 1) * 16]
                                    .rearrange("p c (f a b) -> p c f a b",
                                               a=16, b=2),
                                G[:, kk * 16:(kk + 1) * 16]
                                    .rearrange("p c (f a b) -> p c f a b",
                                               a=16, b=2),
                                wv, OP.mult)
                        if DBG and h == 0 and l == 0 and qh == 0:
                            nc.sync.dma_start(
                                d_dbg_G[:],
                                G[:].rearrange("p a b -> p (a b)"))
                            nc.sync.dma_start(
                                d_dbg_M[:],
                                M[:].rearrange("p a b -> p (a b)"))
                        r1 = gp.tile([128, KPTS * 16, 64], BF16, tag="r1",
                                     name="r1", bufs=2)
                        nc.vector.tensor_tensor(r1[:], M[:, :, 0:64],
                                                M[:, :, 64:128], OP.add)
                        r2 = gp.tile([128, KPTS * 16, DK], BF16, tag="r2",
                                     name="r2", bufs=2)
                        nc.vector.tensor_tensor(r2[:], r1[:, :, 0:32],
                                                r1[:, :, 32:64], OP.add)
                        t1 = gp.tile([128, 2 * 16, DK], BF16, tag="t1",
                                     name="t1", bufs=2)
                        nc.vector.tensor_tensor(t1[:], r2[:, 0:32],
                                                r2[:, 32:64], OP.add)
                        t2 = gp.tile([128, 16, DK], F32, tag="t2", name="t2",
                                     bufs=2)
                        nc.vector.tensor_tensor(t2[:], t1[:, 0:16],
                                                t1[:, 16:32], OP.add)
                        fslice = feat[:, qh * 16:(qh + 1) * 16, h]
                        nc.vector.tensor_tensor(fslice, fslice, t2[:], OP.add)

                        # out[co, h*512+t] = sum_{e,d} Wm[e*32+d, co] *
                        # feat[8t'+e, c, h, d], t = 16c + t'. After the last
                        # scale, this qh-half of feat[·,·,h,·] is final:
                        # transpose it into TS[d, c, p] while gathers continue.
                        if l == SCALES - 1:
                            for c0 in range(qh * 16, qh * 16 + 16, 4):
                                pt2 = psumb.tile([128, 512], F32, tag="tp2",
                                                 name="tp2")
                                for j in range(4):
                                    nc.tensor.transpose(
                                        pt2[0:32, j * 128:(j + 1) * 128],
                                        feat[:, c0 + j, h, :], identF[:])
                                nc.scalar.activation(
                                    TS[0:32, c0:c0 + 4, :]
                                        .rearrange("p c t -> p (c t)"),
                                    pt2[0:32, :], AF.Copy)

                for m in range(2):
                    ps = psum.tile([128, 512], F32, tag="mm", name="mm")
                    for e in range(8):
                        lhsT = WmE[0:32, e, m * 128:(m + 1) * 128]
                        rhs = TS[0:32, :, e:128:8]
                        nc.tensor.matmul(ps[:], lhsT, rhs,
                                         start=(e == 0), stop=(e == 7))
                    outT = op.tile([128, 512], F32, tag=f"oT{m}",
                                   name=f"oT{m}", bufs=2)
                    nc.scalar.activation(outT[:], ps[:], AF.Identity,
                                         bias=bm[m][:], scale=1.0)
                    nc.sync.dma_start(d_out[m][:, h * 512:(h + 1) * 512],
                                      outT[:])

        if DBG:
            tc.strict_bb_all_engine_barrier()
            nc.sync.dma_start(d_dbg_idx[:], IDX16[:].rearrange(
                "p a b c d e f -> p (a b c d e f)"))
            nc.sync.dma_start(d_dbg_feat[:], feat[:].rearrange(
                "p a b c -> p (a b c)"))
            nc.sync.dma_start(d_dbg_w4[:], W4[:].rearrange(
                "p a b c d e f -> p (a b c d e f)"))
            nc.sync.dma_start(d_dbg_map1[:], d_map[1][:])
            nc.sync.dma_start(d_dbg_map0[:], d_map[0][:])

    nc.compile()
    return nc


def _prep_inputs(query, keys, ref_point, Wq, bq, Wk, bk, Woff, boff, WA, bA, Wm, bm):
    def two(w, n):
        return np.ascontiguousarray(w.reshape(2, 128, n).astype(np.float32))

    wl_arr = np.zeros(64, np.float32)
    hl_arr = np.zeros(64, np.float32)
    for h in range(HL):
        for s in range(SCALES):
            for k in range(KPTS):
                hl_, wl_ = HW_SIZES[s]
                wl_arr[h * 16 + s * 4 + k] = wl_
                hl_arr[h * 16 + s * 4 + k] = hl_
    consts = {
        "wl_t": np.tile(wl_arr, (128, 1)),
        "wlm1_t": np.tile(wl_arr - 1, (128, 1)),
        "wlm2_t": np.tile(wl_arr - 2, (128, 1)),
        "wlp1_t": np.tile(wl_arr + 1, (128, 1)),
        "hlm1_t": np.tile(hl_arr - 1, (128, 1)),
        "hlm2_t": np.tile(hl_arr - 2, (128, 1)),
        "hl_t": np.tile(hl_arr, (128, 1)),
        "cofx_t": np.tile(wl_arr / (wl_arr - 1), (128, 1)),
        "cofy_t": np.tile(hl_arr / (hl_arr - 1), (128, 1)),
    }
    consts = {k: np.ascontiguousarray(v.astype(np.float32)) for k, v in consts.items()}

    rs = ref_point.reshape(Q, 2)
    refx = np.ascontiguousarray(rs[:, 0].reshape(QC, 128).T)
    refy = np.ascontiguousarray(rs[:, 1].reshape(QC, 128).T)

    in_maps = []
    for core in range(NCORES):
        b, hg = core // 2, core % 2
        heads = range(4 * hg, 4 * hg + 4)
        perm_off = np.zeros(128, np.int64)
        perm_A = np.zeros(64, np.int64)
        for i, h in enumerate(heads):
            for s in range(SCALES):
                for k in range(KPTS):
                    for xy in range(2):
                        perm_off[xy * 64 + i * 16 + s * 4 + k] = \
                            ((h * SCALES + s) * KPTS + k) * 2 + xy
                    perm_A[i * 16 + s * 4 + k] = (h * SCALES + s) * KPTS + k
        WoffP = np.ascontiguousarray(Woff[:, perm_off])
        boffP = boff[perm_off]
        WAP = np.ascontiguousarray(WA[:, perm_A])
        bAP = bA[perm_A]
        chs = slice(4 * hg * DK, (4 * hg + 4) * DK)
        bq2 = two(bq, 1)
        bm2 = two(bm, 1)
        cblk = np.concatenate(
            [consts[n] for n in ["wl_t", "wlm1_t", "wlm2_t", "wlp1_t",
                                 "hlm1_t", "hlm2_t", "hl_t", "cofx_t",
                                 "cofy_t"]]
            + [np.tile(boffP, (128, 1)).astype(np.float32),
               np.tile(bAP, (128, 1)).astype(np.float32),
               refx, refy,
               bq2[0], bq2[1], bm2[0], bm2[1],
               np.ascontiguousarray(bk[chs]).reshape(128, 1)
               .astype(np.float32)],
            axis=1)
        m = {
            "Wq": two(Wq, D).astype(BFNP),
            "Wk": two(np.ascontiguousarray(Wk[:, chs]), 128).astype(BFNP),
            "WoffP": two(WoffP, 128).astype(BFNP),
            "WA": two(WAP, 64).astype(BFNP),
            "WmE": np.ascontiguousarray(
                Wm.reshape(8, 32, D).transpose(1, 0, 2)).astype(BFNP),
            "cblk": np.ascontiguousarray(cblk),
        }
        qs = query[b].reshape(Q, D)
        m["queryT"] = np.ascontiguousarray(qs.T).reshape(2, 128, Q).astype(BFNP)
        for l in range(SCALES):
            m[f"keysT{l}"] = np.ascontiguousarray(
                keys[l][b].reshape(POS[l], D).T).reshape(2, 128, POS[l]) \
                .astype(BFNP)
        in_maps.append(m)
    return in_maps


def kernel(query, keys0, keys1, keys2, keys3, ref_point,
           Wq, bq, Wk, bk, Woff, boff, WA, bA, Wm, bm):
    query = np.asarray(query, np.float32)
    keys = [np.asarray(k, np.float32) for k in (keys0, keys1, keys2, keys3)]
    in_maps = _prep_inputs(
        query, keys, np.asarray(ref_point, np.float32),
        np.asarray(Wq, np.float32), np.asarray(bq, np.float32),
        np.asarray(Wk, np.float32), np.asarray(bk, np.float32),
        np.asarray(Woff, np.float32), np.asarray(boff, np.float32),
        np.asarray(WA, np.float32), np.asarray(bA, np.float32),
        np.asarray(Wm, np.float32), np.asarray(bm, np.float32))
    if "nc" not in _cache:
        _cache["nc"] = _build()
    nc = _cache["nc"]
    res = run_bass_kernel_spmd(nc, in_maps, list(range(NCORES)))
    out = np.zeros((B, H, W, D), np.float32)
    for core in range(NCORES):
        b, hg = core // 2, core % 2
        oT = res.results[core]["outT"].reshape(D, 2048)
        out[b, 32 * hg:32 * hg + 32] = oT.T.reshape(32, W, D)
    return out


# revision 94
# speedup vs baseline: 1.0019x; 1.0019x over previous
"""Deformable head attention kernel for 8 Trainium2 NeuronCores.

Sharding: core i handles batch b = i//2 and head-group hg = i%2 (heads
4*hg..4*hg+3, all 4096 queries). The reference's final reshape maps output
pixel p' to head p'//512's features of queries 8t..8t+7 (t = p' % 512), so a
head-group owns output rows [hg*32, hg*32+32) exactly -- fully local per core.

Per core (v3, overlap-oriented; ~650us vs the 1107us v1 baseline):
  - bf16 host-prepped inputs/weights (halves the key loads, 1-cycle matmuls)
  - per-(head,scale) quad maps (2x2x32ch tokens, 256B bf16) in DRAM; key
    slabs prefetched one ahead so the write stream never stalls; pad regions
    zero-filled byte-disjointly (no WAW ordering needed) and deferred off the
    early HWDGE window
  - gather indices built with PE "selection matmuls" (ident column slices
    fold partition ph*16+r to idx row r with ph in the free dim — the SWDGE
    16-wrap layout) instead of 256 small DMAs; idx rows replicated to all
    eight 16-partition groups per q-half (the gather engines read each group)
  - coordinate/weight math split x-on-DVE / y-on-Pool, quarter-tiled for SBUF
  - SWDGE dma_gather fetches one quad per sample (the 22.76ns/descriptor
    cost-model floor); the weighted reduce runs on DVE so the 32-gather train
    is purely DMA-bound at ~99% occupancy
  - output projection straight from SBUF: per (head, q-half), feat slices are
    PE-transposed to TS[d, c, p] as their last scale lands, then eight
    32-partition Wm row-block matmuls (column slice p=e::8) accumulate the
    e-partials in PSUM — no DRAM scramble round-trip and no tail barrier
"""
import os
import numpy as np
import ml_dtypes
from contextlib import ExitStack

BFNP = ml_dtypes.bfloat16

import concourse.bass as bass
import concourse.tile as tile
from concourse import bacc, mybir
from concourse.bass_utils import run_bass_kernel_spmd
from concourse.masks import make_identity

F32 = mybir.dt.float32
F32R = mybir.dt.float32r
I32 = mybir.dt.int32
I16 = mybir.dt.int16
BF16 = mybir.dt.bfloat16
OP = mybir.AluOpType
AF = mybir.ActivationFunctionType

HEADS, KPTS, SCALES, D = 8, 4, 4, 256
DK = D // HEADS              # 32
HL = 4                       # heads per core
B, H, W = 4, 64, 64
Q = 4096                     # queries per core (full image)
QC = Q // 128                # 32 q-chunks
HW_SIZES = [(16, 16), (32, 32), (64, 64), (128, 128)]
POS = [h * w for h, w in HW_SIZES]
TCAP = [p + w + 4 for p, (h, w) in zip(POS, HW_SIZES)]
NCORES = 8
NIDX = KPTS * 2048           # 8192 gather indices per (head, scale, q-half)
LS = 2048                    # key-projection slab (tokens)

_cache = {}


def _build():
    nc = bacc.Bacc("TRN2", target_bir_lowering=False, debug=False)

    d_queryT = nc.dram_tensor("queryT", [2, 128, Q], BF16, kind="ExternalInput")
    d_keysT = [nc.dram_tensor(f"keysT{l}", [2, 128, POS[l]], BF16,
                              kind="ExternalInput")
               for l in range(SCALES)]
    d_cblk = nc.dram_tensor("cblk", [128, 837], F32, kind="ExternalInput")
    d_Wq = nc.dram_tensor("Wq", [2, 128, D], BF16, kind="ExternalInput")
    d_Wk = nc.dram_tensor("Wk", [2, 128, 128], BF16, kind="ExternalInput")
    d_Woff = nc.dram_tensor("WoffP", [2, 128, 128], BF16, kind="ExternalInput")
    d_WA = nc.dram_tensor("WA", [2, 128, 64], BF16, kind="ExternalInput")
    d_Wm = nc.dram_tensor("WmE", [32, 8, D], BF16, kind="ExternalInput")
    cnames = ["wl_t", "wlm1_t", "wlm2_t", "wlp1_t", "hlm1_t", "hlm2_t",
              "hl_t", "cofx_t", "cofy_t"]

    d_out = nc.dram_tensor("outT", [2, 128, 2048], F32, kind="ExternalOutput")
    DBG = bool(int(os.environ.get("KDBG", "0")))
    if DBG:
        d_dbg_G = nc.dram_tensor("dbg_G", [128, 8192], BF16,
                                 kind="ExternalOutput")
        d_dbg_M = nc.dram_tensor("dbg_M", [128, 8192], BF16,
                                 kind="ExternalOutput")
        d_dbg_idx = nc.dram_tensor("dbg_idx", [128, 16384], I16,
                                   kind="ExternalOutput")
        d_dbg_feat = nc.dram_tensor("dbg_feat", [128, 4096], F32,
                                    kind="ExternalOutput")
        d_dbg_w4 = nc.dram_tensor("dbg_w4", [128, 16384], BF16,
                                  kind="ExternalOutput")
        d_dbg_map1 = nc.dram_tensor("dbg_map1", [HL, TCAP[1], 128], BF16,
                                    kind="ExternalOutput")
        d_dbg_map0 = nc.dram_tensor("dbg_map0", [HL, TCAP[0], 128], BF16,
                                    kind="ExternalOutput")

    with tile.TileContext(nc) as tc, ExitStack() as ctx:
        wpool = ctx.enter_context(tc.tile_pool(name="weights", bufs=1))
        ppool = ctx.enter_context(tc.tile_pool(name="persist", bufs=1))
        psum = ctx.enter_context(tc.tile_pool(name="psum", bufs=4, space="PSUM"))
        psumb = ctx.enter_context(tc.tile_pool(name="psumb", bufs=2, space="PSUM"))
        dram = ctx.enter_context(tc.tile_pool(name="dramp", bufs=1, space="DRAM"))

        d_map = [dram.tile([HL, TCAP[l], 128], BF16, tag=f"map{l}", name=f"map{l}")
                 for l in range(SCALES)]
        d_featD = dram.tile([2048, 256], F32, tag="featD", name="featD")

        def load2(d, n, nm, dt=F32):
            t2 = wpool.tile([128, 2, n], dt, tag=nm, name=nm)
            nc.sync.dma_start(t2[:], d[:].rearrange("k p n -> p k n"))
            return [t2[:, 0], t2[:, 1]]

        def load1(d, shape, nm):
            t = wpool.tile(shape, F32, tag=nm, name=nm)
            nc.sync.dma_start(t[:], d[:])
            return t

        Wq = load2(d_Wq, D, "Wq", BF16); Wk = load2(d_Wk, 128, "Wk", BF16)
        Woff = load2(d_Woff, 128, "Woff", BF16)
        WA = load2(d_WA, 64, "WA", BF16)
        WmE = wpool.tile([128, 8, D], BF16, tag="WmE", name="WmE")
        nc.sync.dma_start(WmE[0:32], d_Wm[:])
        cblk = load1(d_cblk, [128, 837], "cblk")
        C = {n: cblk[:, i * 64:(i + 1) * 64] for i, n in enumerate(cnames)}
        boffE = cblk[:, 576:704]
        bAE = cblk[:, 704:768]
        refx = cblk[:, 768:800]
        refy = cblk[:, 800:832]
        bq = [cblk[:, 832 + i:833 + i] for i in range(2)]
        bm = [cblk[:, 834 + i:835 + i] for i in range(2)]
        bk = cblk[:, 836:837]
        identF = wpool.tile([128, 128], F32, tag="identF", name="identF")
        make_identity(nc, identF[:])
        identB = wpool.tile([128, 128], BF16, tag="identB", name="identB")
        make_identity(nc, identB[:])
        zt = wpool.tile([128, HL, 32], BF16, tag="zt", name="zt")
        nc.vector.memset(zt[:], 0)

        # persistent state
        W4 = ppool.tile([128, HL, SCALES, KPTS, QC, 4, 2], BF16, tag="W4",
                        name="W4")
        feat = ppool.tile([128, QC, HL, DK], F32, tag="feat", name="feat")
        IDX16 = ppool.tile([128, 2, HL, SCALES, KPTS, 16, 8], I16,
                           tag="IDX16", name="IDX16")
        nc.vector.memset(IDX16[:], 0)
        nc.vector.memset(feat[:], 0)

        # =========== quad map construction (phase D) ===========
        mp = tc.alloc_tile_pool(name="mapp", bufs=1)
        CORNERS = [(0, 0), (0, 1), (1, 0), (1, 1)]

        def zero_edges(l):
            """Fill every map byte not covered by a corner write with zeros.
            Corner (cy,cx) covers entries [base, base+POS) at entry-offset
            (cy*2+cx)*32; zero the complement so reads never see garbage."""
            mapl = d_map[l]
            hl_, wl = HW_SIZES[l]
            for cy, cx in CORNERS:
                base = wl + 1 - cy * wl - cx
                off = (cy * 2 + cx) * 32
                for t0, t1 in ((0, base), (base + POS[l], TCAP[l])):
                    t = t0
                    while t < t1:
                        n = min(128, t1 - t)
                        dst = bass.AP(
                            mapl.tensor,
                            mapl.offset + t * 128 + off,
                            ((128, n), (TCAP[l] * 128, HL), (1, 32)))
                        nc.scalar.dma_start(dst, zt[0:n, :, :])
                        t += n

        def load_slab(l, sl, slab):
            kin = [mp.tile([128, slab], BF16, tag=f"kin{i}", name=f"kin{i}",
                           bufs=2) for i in range(2)]
            for i in range(2):
                nc.sync.dma_start(kin[i][:],
                                  d_keysT[l][i, :, sl * slab:(sl + 1) * slab])
            return kin

        def build_map(l):
            hl_, wl = HW_SIZES[l]
            mapl = d_map[l]
            slab = min(LS, POS[l])
            nslab = POS[l] // slab
            kin = load_slab(l, 0, slab)
            for sl in range(nslab):
                kin_next = load_slab(l, sl + 1, slab) if sl + 1 < nslab else None
                kfs = mp.tile([128, slab], BF16, tag="kfs", name="kfs", bufs=2)
                for c0 in range(0, slab, 512):
                    cw = min(512, slab - c0)
                    ps = psum.tile([128, 512], F32, tag="mm", name="mm")
                    for k in range(2):
                        nc.tensor.matmul(ps[:, 0:cw], Wk[k][:],
                                         kin[k][:, c0:c0 + cw],
                                         start=(k == 0), stop=(k == 1))
                    nc.scalar.activation(kfs[:, c0:c0 + cw], ps[:, 0:cw],
                                         AF.Identity, bias=bk[:], scale=1.0)
                nsub = slab // 128
                stg = mp.tile([128, nsub, 128], BF16, tag="stg", name="stg",
                              bufs=3)
                for g0 in range(0, nsub, 4):
                    gw = min(4, nsub - g0)
                    pt = psumb.tile([128, 512], BF16, tag="tp", name="tp")
                    for j in range(gw):
                        nc.tensor.transpose(
                            pt[:, j * 128:(j + 1) * 128],
                            kfs[:, (g0 + j) * 128:(g0 + j + 1) * 128],
                            identB[:])
                    nc.scalar.activation(
                        stg[:, g0:g0 + gw].rearrange("p a e -> p (a e)"),
                        pt[:, 0:gw * 128], AF.Copy)
                src = stg[:].rearrange("p a (h e) -> p h a e", e=DK)
                for cy, cx in CORNERS:
                    base = wl + 1 - cy * wl - cx
                    off = (base + sl * slab) * 128 + (cy * 2 + cx) * DK
                    for hh in range(HL):
                        dst = bass.AP(
                            mapl.tensor,
                            mapl.offset + off + hh * TCAP[l] * 128,
                            ((128, 128), (128 * 128, nsub), (1, DK)))
                        nc.sync.dma_start(dst, src[:, hh])
                kin = kin_next

        # queryT loads ride the Act queue so the SP/map DMA stream is unbroken
        pbc = tc.alloc_tile_pool(name="pbc", bufs=1)
        Aw = pbc.tile([128, QC, 64], F32, tag="Aw", name="Aw")
        offxy = pbc.tile([128, QC, 128], F32, tag="offxy", name="offxy")
        proj = tc.alloc_tile_pool(name="proj", bufs=1)
        queryT = [proj.tile([128, Q], BF16, tag=f"qin{i}", name=f"qin{i}")
                  for i in range(2)]
        for i in range(2):
            nc.scalar.dma_start(queryT[i][:], d_queryT[i])

        for l in range(3):
            build_map(l)

        # =========== phase B: q / offset / attention projections ===========
        if True:
            qT = [proj.tile([128, Q], BF16, tag=f"qT{i}", name=f"qT{i}")
                  for i in range(2)]
            for m in range(2):
                for n in range(Q // 512):
                    ps = psum.tile([128, 512], F32, tag="mm", name="mm")
                    for k in range(2):
                        nc.tensor.matmul(ps[:], Wq[k][:, m * 128:(m + 1) * 128],
                                         queryT[k][:, n * 512:(n + 1) * 512],
                                         start=(k == 0), stop=(k == 1))
                    nc.scalar.activation(qT[m][:, n * 512:(n + 1) * 512], ps[:],
                                         AF.Identity, bias=bq[m][:], scale=1.0)
            for c in range(QC):
                ps = psum.tile([128, 512], F32, tag="mm", name="mm")
                for k in range(2):
                    nc.tensor.matmul(ps[:, 0:128], qT[k][:, c * 128:(c + 1) * 128],
                                     Woff[k][:], start=(k == 0), stop=(k == 1))
                nc.scalar.activation(offxy[:, c], ps[:, 0:128], AF.Copy)
                ps2 = psum.tile([128, 512], F32, tag="mm", name="mm")
                for k in range(2):
                    nc.tensor.matmul(ps2[:, 0:64], qT[k][:, c * 128:(c + 1) * 128],
                                     WA[k][:], start=(k == 0), stop=(k == 1))
                nc.scalar.activation(Aw[:, c], ps2[:, 0:64], AF.Copy)
        proj.release()

        # =========== phase C: coords, weights, tokens (x: DVE, y: Pool) ====
        tokp = tc.alloc_tile_pool(name="tokp", bufs=1)
        tokf = [tokp.tile([128, 16, 64], F32, tag=f"tokf{qh}", name=f"tokf{qh}")
                for qh in range(2)]
        NQ = 8                   # q-chunks per coord quarter

        def bhq(t, n=64):
            return t[:].rearrange("p (o f) -> p o f", o=1).broadcast_to([128, NQ, n])

        for qq in range(QC // NQ):
            sl = slice(qq * NQ, (qq + 1) * NQ)
            with tc.tile_pool(name=f"coord{qq}", bufs=1) as cp:
                def ct(tag, dt=F32):
                    return cp.tile([128, NQ, 64], dt, tag=tag, name=tag)

                ix = ct("ix"); iy = ct("iy")
                for c in range(NQ):
                    cc = qq * NQ + c
                    nc.vector.tensor_scalar(ix[:, c], C["wl_t"][:],
                                            refx[:, cc:cc + 1], -0.5,
                                            OP.mult, OP.add)
                    nc.gpsimd.tensor_scalar(iy[:, c], C["hl_t"][:],
                                            refy[:, cc:cc + 1], -0.5,
                                            OP.mult, OP.add)
                oxv = offxy[:, sl, 0:64]
                oyv = offxy[:, sl, 64:128]
                nc.vector.tensor_tensor(oxv, oxv, bhq(boffE[:, 0:64]), OP.add)
                nc.gpsimd.tensor_tensor(oyv, oyv, bhq(boffE[:, 64:128]), OP.add)
                nc.vector.tensor_tensor(oxv, oxv, bhq(C["cofx_t"]), OP.mult)
                nc.gpsimd.tensor_tensor(oyv, oyv, bhq(C["cofy_t"]), OP.mult)
                nc.vector.tensor_tensor(ix[:], ix[:], oxv, OP.add)
                nc.gpsimd.tensor_tensor(iy[:], iy[:], oyv, OP.add)

                def dimchain(eng, ceng, iv, lim1, lim2, pre):
                    # ceng runs the comparison (bitvec) ops: Pool lacks them
                    xm = ct(f"xm{pre}")
                    eng.tensor_scalar(xm[:], iv[:], 0.5, None, OP.subtract)
                    xi = ct(f"xi{pre}", I32)
                    eng.tensor_copy(xi[:], xm[:])      # RNE => floor(iv)
                    x0 = ct(f"x0{pre}")
                    eng.tensor_copy(x0[:], xi[:])
                    w1 = ct(f"w1{pre}")
                    eng.tensor_tensor(w1[:], iv[:], x0[:], OP.subtract)
                    w0 = ct(f"w0{pre}")
                    eng.tensor_scalar(w0[:], w1[:], -1.0, 1.0, OP.mult, OP.add)
                    m = ct(f"m{pre}")
                    mb = ct(f"mb{pre}")
                    ceng.tensor_scalar(m[:], x0[:], 0.0, None, OP.is_ge)
                    ceng.tensor_tensor(mb[:], x0[:], bhq(lim1), OP.is_le)
                    eng.tensor_tensor(m[:], m[:], mb[:], OP.mult)
                    eng.tensor_tensor(w0[:], w0[:], m[:], OP.mult)
                    ceng.tensor_scalar(m[:], x0[:], -1.0, None, OP.is_ge)
                    ceng.tensor_tensor(mb[:], x0[:], bhq(lim2), OP.is_le)
                    eng.tensor_tensor(m[:], m[:], mb[:], OP.mult)
                    eng.tensor_tensor(w1[:], w1[:], m[:], OP.mult)
                    return x0, w0, w1

                x0f, wx0, wx1 = dimchain(nc.vector, nc.vector, ix,
                                         C["wlm1_t"], C["wlm2_t"], "x")
                y0f, wy0, wy1 = dimchain(nc.gpsimd, nc.vector, iy,
                                         C["hlm1_t"], C["hlm2_t"], "y")

                # softmax over (s, k) per head for this q-quarter
                Av = Aw[:, sl]
                nc.vector.tensor_tensor(Av, Av, bhq(bAE), OP.add)
                nc.scalar.activation(Av, Av, AF.Exp)
                Aw4 = Av.rearrange("p c (h s) -> p c h s", s=16)
                ssum = cp.tile([128, NQ, HL], F32, tag="ssum", name="ssum")
                nc.vector.tensor_reduce(ssum[:], Aw4, mybir.AxisListType.X,
                                        OP.add)
                nc.vector.reciprocal(ssum[:], ssum[:])
                rb = ssum[:].rearrange("p c (h o) -> p c h o", o=1) \
                            .broadcast_to([128, NQ, HL, 16])
                nc.vector.tensor_tensor(Aw4, Aw4, rb, OP.mult)

                # combined interp weights
                W4v = W4[:].rearrange("p h s k c f d -> p c (h s k) f d")[:, sl]
                u = ct("mx")      # reuse
                for cy, wyv in ((0, wy0), (1, wy1)):
                    nc.vector.tensor_tensor(u[:], Av, wyv[:], OP.mult)
                    for cx, wxv in ((0, wx0), (1, wx1)):
                        for dup in range(2):
                            eng = nc.vector if (cx + dup) % 2 == 0 else nc.gpsimd
                            eng.tensor_tensor(
                                W4v[:, :, :, cy * 2 + cx, dup],
                                u[:], wxv[:], OP.mult)

                # token index (+wl+1 offset baked in); min/max live on DVE
                nc.vector.tensor_scalar(y0f[:], y0f[:], -1.0, None, OP.max)
                nc.vector.tensor_tensor(y0f[:], y0f[:], bhq(C["hlm1_t"]),
                                        OP.min)
                nc.vector.tensor_scalar(x0f[:], x0f[:], -1.0, None, OP.max)
                nc.vector.tensor_tensor(x0f[:], x0f[:], bhq(C["wl_t"]), OP.min)
                tk = tokf[qq // 2][:, (qq % 2) * NQ:(qq % 2 + 1) * NQ]
                nc.gpsimd.tensor_tensor(tk, y0f[:], bhq(C["wl_t"]), OP.mult)
                nc.gpsimd.tensor_tensor(tk, tk, x0f[:], OP.add)
                nc.gpsimd.tensor_tensor(tk, tk, bhq(C["wlp1_t"]), OP.add)

        build_map(3)
        # pad zero-fill is byte-disjoint from the corner writes, so it can
        # run this late — keeping early HWDGE free for the map pipeline
        for l in range(SCALES):
            zero_edges(l)

        # =========== gather-index build: PE partition fold ===========
        # idx[r, (k, qcl, ph)] = token[p = ph*16 + r]; selection matmul with
        # an identity column slice folds partitions exactly into the SWDGE
        # 16-wrap layout. f32 matmul is exact for these integer magnitudes.
        with tc.tile_pool(name="idxb", bufs=1) as ib:
            for qh in range(2):
                tsrc = tokf[qh][:].rearrange("p c h -> p (c h)")
                for ph in range(8):
                    for ck in range(2):
                        ps = psum.tile([128, 512], F32, tag="mm", name="mm")
                        nc.tensor.matmul(ps[0:16, :],
                                         identF[:, ph * 16:ph * 16 + 16],
                                         tsrc[:, ck * 512:(ck + 1) * 512],
                                         start=True, stop=True)
                        sti = ib.tile([128, 512], I32, tag="sti", name="sti",
                                      bufs=2)
                        nc.vector.tensor_copy(sti[0:16, :], ps[0:16, :])
                        dst = IDX16[0:16, qh, :, :, :, ck * 8:(ck + 1) * 8, ph]
                        src = sti[0:16].bitcast(I16)[:, 0:1024:2] \
                            .rearrange("p (c h s k) -> p h s k c",
                                       c=8, h=HL, s=SCALES)
                        nc.gpsimd.tensor_copy(dst, src)
                # the gather engines read idx from every 16-partition group:
                # replicate rows 0:16 across all 128 as each half finishes
                for d0, n in ((16, 16), (32, 32), (64, 64)):
                    nc.sync.dma_start(IDX16[d0:d0 + n, qh], IDX16[0:n, qh])
        tokp.release()
        pbc.release()
        mp.release()

        # DRAM tiles get no RAW tracking into dma_gather: fence map writes
        # (and IDX16) before the gather train starts.
        tc.strict_bb_all_engine_barrier()

        # =========== phase E: gather + interpolate, per-head output =======
        op = ctx.enter_context(tc.tile_pool(name="outp", bufs=1))
        fD = d_featD[:].rearrange("o c -> (o c)")
        with tc.tile_pool(name="gath", bufs=1) as gp:
            for h in range(HL):
                TS = op.tile([128, QC, 128], BF16, tag="TS", name="TS",
                             bufs=1)
                for l in range(SCALES):
                    for qh in range(2):
                        G = gp.tile([128, KPTS * 16, 128], BF16, tag="G",
                                    name="G", bufs=3)
                        nc.gpsimd.dma_gather(
                            G[:], d_map[l][h],
                            IDX16[:, qh, h, l].rearrange("p k c e -> p (k c e)"),
                            num_idxs=NIDX, num_idxs_reg=NIDX,
                            elem_size=128, elem_step=128, single_packet=False)
                        M = gp.tile([128, KPTS * 16, 128], BF16, tag="M",
                                    name="M", bufs=1)
                        for kk in range(KPTS):
                            wv = W4[:, h, l, kk, qh * 16:(qh + 1) * 16, :, :] \
                                .rearrange("p c f (o d) -> p c f o d", o=1) \
                                .broadcast_to([128, 16, 4, 16, 2])
                            nc.vector.tensor_tensor(
                                M[:, kk * 16:(kk + 1) * 16]
                                    .rearrange("p c (f a b) -> p c f a b",
                                               a=16, b=2),
                                G[:, kk * 16:(kk + 1) * 16]
                                    .rearrange("p c (f a b) -> p c f a b",
                                               a=16, b=2),
                                wv, OP.mult)
                        if DBG and h == 0 and l == 0 and qh == 0:
                            nc.sync.dma_start(
                                d_dbg_G[:],
                                G[:].rearrange("p a b -> p (a b)"))
                            nc.sync.dma_start(
                                d_dbg_M[:],
                                M[:].rearrange("p a b -> p (a b)"))
                        r1 = gp.tile([128, KPTS * 16, 64], BF16, tag="r1",
                                     name="r1", bufs=2)
                        nc.vector.tensor_tensor(r1[:], M[:, :, 0:64],
                                                M[:, :, 64:128], OP.add)
                        r2 = gp.tile([128, KPTS * 16, DK], BF16, tag="r2",
                                     name="r2", bufs=2)
                        nc.vector.tensor_tensor(r2[:], r1[:, :, 0:32],
                                                r1[:, :, 32:64], OP.add)
                        t1 = gp.tile([128, 2 * 16, DK], BF16, tag="t1",
                                     name="t1", bufs=2)
                        nc.vector.tensor_tensor(t1[:], r2[:, 0:32],
                                                r2[:, 32:64], OP.add)
                        t2 = gp.tile([128, 16, DK], F32, tag="t2", name="t2",
                                     bufs=2)
                        nc.vector.tensor_tensor(t2[:], t1[:, 0:16],
                                                t1[:, 16:32], OP.add)
                        fslice = feat[:, qh * 16:(qh + 1) * 16, h]
                        nc.vector.tensor_tensor(fslice, fslice, t2[:], OP.add)

                        # out[co, h*512+t] = sum_{e,d} Wm[e*32+d, co] *
                        # feat[8t'+e, c, h, d], t = 16c + t'. After the last
                        # scale, this qh-half of feat[·,·,h,·] is final:
                        # transpose it into TS[d, c, p] while gathers continue.
                        if l == SCALES - 1:
                            for c0 in range(qh * 16, qh * 16 + 16, 4):
                                pt2 = psumb.tile([128, 512], F32, tag="tp2",
                                                 name="tp2")
                                for j in range(4):
                                    nc.tensor.transpose(
                                        pt2[0:32, j * 128:(j + 1) * 128],
                                        feat[:, c0 + j, h, :], identF[:])
                                nc.scalar.activation(
                                    TS[0:32, c0:c0 + 4, :]
                                        .rearrange("p c t -> p (c t)"),
                                    pt2[0:32, :], AF.Copy)

                for m in range(2):
                    ps = psum.tile([128, 512], F32, tag="mm", name="mm")
                    for e in range(8):
                        lhsT = WmE[0:32, e, m * 128:(m + 1) * 128]
                        rhs = TS[0:32, :, e:128:8]
                        nc.tensor.matmul(ps[:], lhsT, rhs,
                                         start=(e == 0), stop=(e == 7))
                    outT = op.tile([128, 512], F32, tag=f"oT{m}",
                                   name=f"oT{m}", bufs=2)
                    nc.scalar.activation(outT[:], ps[:], AF.Identity,
                                         bias=bm[m][:], scale=1.0)
                    nc.sync.dma_start(d_out[m][:, h * 512:(h + 1) * 512],
                                      outT[:])

        if DBG:
            tc.strict_bb_all_engine_barrier()
            nc.sync.dma_start(d_dbg_idx[:], IDX16[:].rearrange(
                "p a b c d e f -> p (a b c d e f)"))
            nc.sync.dma_start(d_dbg_feat[:], feat[:].rearrange(
                "p a b c -> p (a b c)"))
            nc.sync.dma_start(d_dbg_w4[:], W4[:].rearrange(
                "p a b c d e f -> p (a b c d e f)"))
            nc.sync.dma_start(d_dbg_map1[:], d_map[1][:])
            nc.sync.dma_start(d_dbg_map0[:], d_map[0][:])

    nc.compile()
    return nc


def _prep_inputs(query, keys, ref_point, Wq, bq, Wk, bk, Woff, boff, WA, bA, Wm, bm):
    def two(w, n):
        return np.ascontiguousarray(w.reshape(2, 128, n).astype(np.float32))

    wl_arr = np.zeros(64, np.float32)
    hl_arr = np.zeros(64, np.float32)
    for h in range(HL):
        for s in range(SCALES):
            for k in range(KPTS):
                hl_, wl_ = HW_SIZES[s]
                wl_arr[h * 16 + s * 4 + k] = wl_
                hl_arr[h * 16 + s * 4 + k] = hl_
    consts = {
        "wl_t": np.tile(wl_arr, (128, 1)),
        "wlm1_t": np.tile(wl_arr - 1, (128, 1)),
        "wlm2_t": np.tile(wl_arr - 2, (128, 1)),
        "wlp1_t": np.tile(wl_arr + 1, (128, 1)),
        "hlm1_t": np.tile(hl_arr - 1, (128, 1)),
        "hlm2_t": np.tile(hl_arr - 2, (128, 1)),
        "hl_t": np.tile(hl_arr, (128, 1)),
        "cofx_t": np.tile(wl_arr / (wl_arr - 1), (128, 1)),
        "cofy_t": np.tile(hl_arr / (hl_arr - 1), (128, 1)),
    }
    consts = {k: np.ascontiguousarray(v.astype(np.float32)) for k, v in consts.items()}

    rs = ref_point.reshape(Q, 2)
    refx = np.ascontiguousarray(rs[:, 0].reshape(QC, 128).T)
    refy = np.ascontiguousarray(rs[:, 1].reshape(QC, 128).T)

    in_maps = []
    for core in range(NCORES):
        b, hg = core // 2, core % 2
        heads = range(4 * hg, 4 * hg + 4)
        perm_off = np.zeros(128, np.int64)
        perm_A = np.zeros(64, np.int64)
        for i, h in enumerate(heads):
            for s in range(SCALES):
                for k in range(KPTS):
                    for xy in range(2):
                        perm_off[xy * 64 + i * 16 + s * 4 + k] = \
                            ((h * SCALES + s) * KPTS + k) * 2 + xy
                    perm_A[i * 16 + s * 4 + k] = (h * SCALES + s) * KPTS + k
        WoffP = np.ascontiguousarray(Woff[:, perm_off])
        boffP = boff[perm_off]
        WAP = np.ascontiguousarray(WA[:, perm_A])
        bAP = bA[perm_A]
        chs = slice(4 * hg * DK, (4 * hg + 4) * DK)
        bq2 = two(bq, 1)
        bm2 = two(bm, 1)
        cblk = np.concatenate(
            [consts[n] for n in ["wl_t", "wlm1_t", "wlm2_t", "wlp1_t",
                                 "hlm1_t", "hlm2_t", "hl_t", "cofx_t",
                                 "cofy_t"]]
            + [np.tile(boffP, (128, 1)).astype(np.float32),
               np.tile(bAP, (128, 1)).astype(np.float32),
               refx, refy,
               bq2[0], bq2[1], bm2[0], bm2[1],
               np.ascontiguousarray(bk[chs]).reshape(128, 1)
               .astype(np.float32)],
            axis=1)
        m = {
            "Wq": two(Wq, D).astype(BFNP),
            "Wk": two(np.ascontiguousarray(Wk[:, chs]), 128).astype(BFNP),
            "WoffP": two(WoffP, 128).astype(BFNP),
            "WA": two(WAP, 64).astype(BFNP),
            "WmE": np.ascontiguousarray(
                Wm.reshape(8, 32, D).transpose(1, 0, 2)).astype(BFNP),
            "cblk": np.ascontiguousarray(cblk),
        }
        qs = query[b].reshape(Q, D)
        m["queryT"] = np.ascontiguousarray(qs.T).reshape(2, 128, Q).astype(BFNP)
        for l in range(SCALES):
            m[f"keysT{l}"] = np.ascontiguousarray(
                keys[l][b].reshape(POS[l], D).T).reshape(2, 128, POS[l]) \
                .astype(BFNP)
        in_maps.append(m)
    return in_maps


def kernel(query, keys0, keys1, keys2, keys3, ref_point,
           Wq, bq, Wk, bk, Woff, boff, WA, bA, Wm, bm):
    query = np.asarray(query, np.float32)
    keys = [np.asarray(k, np.float32) for k in (keys0, keys1, keys2, keys3)]
    in_maps = _prep_inputs(
        query, keys, np.asarray(ref_point, np.float32),
        np.asarray(Wq, np.float32), np.asarray(bq, np.float32),
        np.asarray(Wk, np.float32), np.asarray(bk, np.float32),
        np.asarray(Woff, np.float32), np.asarray(boff, np.float32),
        np.asarray(WA, np.float32), np.asarray(bA, np.float32),
        np.asarray(Wm, np.float32), np.asarray(bm, np.float32))
    if "nc" not in _cache:
        _cache["nc"] = _build()
    nc = _cache["nc"]
    res = run_bass_kernel_spmd(nc, in_maps, list(range(NCORES)))
    out = np.zeros((B, H, W, D), np.float32)
    for core in range(NCORES):
        b, hg = core // 2, core % 2
        oT = res.results[core]["outT"].reshape(D, 2048)
        out[b, 32 * hg:32 * hg + 32] = oT.T.reshape(32, W, D)
    return out
